# revision 68
# baseline (speedup 1.0000x reference)
"""Trainium2 Bass kernel for AttentionalAggregation (segment softmax-weighted sum).

reference math:
    s = values @ gate_w + gate_b            # [N,1]
    w = segment_softmax(s, indices)         # [N,1]
    out = segment_sum(w * (values @ attn_w + attn_b))   # [G,EMB]

Algebraic restructuring (exact up to fp rounding):
  softmax weights per segment sum to 1, so
      out[g] = (U[g]/D[g]) @ attn_w + attn_b
  with U[g] = sum_{i in g} e_i * values_i, D[g] = sum_{i in g} e_i,
  e_i = exp(values_i . gate_w).  gate_b and the per-segment max shift
  cancel in the U/D ratio (|s| <= ~4 for this data, exp can't overflow).

Sharding: indices are sorted, so each of the 8 cores owns G/8 contiguous
segments and their (contiguous) nodes. No collectives.

This version streams `values` in bf16 (the 2e-2 relative-error budget has
plenty of room: bf16 rounding contributes ~3e-3), which halves HBM traffic
and runs the PE at 1 cycle/row instead of fp32's 4.  Each value row is
augmented with a trailing 1.0 column so ONE matmul per 128-node block
yields both U (cols 0:256) and D (col 256).  The per-node one-hot segment
masks are precomputed on the host (they are a re-encoding of `indices`)
and DMA'd in bf16 (SEGW/EMB ~ 6% extra bytes), so the device-side
per-group (16-block) work is:
  - DVE tensor_mul (packed bf16, 2x mode): prod = v * gate, batched
  - the per-block free-dim reduces s[p] = sum_j prod[p,j] split between
    the DVE (batched tensor_reduce, 11/16) and ACT (Copy+accum_out, 5/16)
    -- measured on HW these balance at ~5us/group each
  - ACT exp (one batched op), GPSIMD broadcast-multiply P_e = onehot * e
  - PE matmul per block (accumulated over the window's blocks in PSUM):
        uw[0:SEGW, 0:257] += P_e.T @ [v | 1]
The window epilogue transposes
uw[.., 0:256] into per-core staging tiles and copies the D column; the
final phase computes Z = U @ attn_w + D*attn_b with 3 matmuls per
128-segment group and scales by 1/D via ACT per-partition scale (D is
rearranged into per-partition layout by a tiny DRAM round-trip).

Everything is static: no sequencer registers, no dynamic access patterns.
"""

import numpy as np
import ml_dtypes

P = 128
EMB = 256
EMBA = EMB + 1      # v rows augmented with a ones column
HALF = 128
SEGW = 16           # segments per window == one-hot width
ACT_FRAC16 = 6      # blocks per 16 whose gate-dot reduce runs on ACT
NCORES = 8
BLK_PER_DMA = 16    # blocks per DMA group
GRP = 128           # segments per final-matmul group

_CACHE = {}


# ----------------------------------------------------------------------------
# Host-side preparation: shard + pad nodes into (core, window, block) layout.
# ----------------------------------------------------------------------------
def prepare_host(values, indices, G):
    idx = np.ascontiguousarray(np.asarray(indices).astype(np.int64))
    counts = np.bincount(idx, minlength=G)
    seg_start = np.zeros(G + 1, dtype=np.int64)
    np.cumsum(counts, out=seg_start[1:])

    assert G % NCORES == 0
    spc = G // NCORES                      # segments per core
    win_lo = list(range(0, spc, SEGW))     # window seg offsets within a core
    win_w = [min(SEGW, spc - lo) for lo in win_lo]
    W = len(win_lo)

    # blocks per window index = max over cores (SPMD: one program, 8 cores)
    b_w = []
    for w in range(W):
        need = 1
        for c in range(NCORES):
            s0 = c * spc + win_lo[w]
            n = int(seg_start[s0 + win_w[w]] - seg_start[s0])
            need = max(need, (n + P - 1) // P)
        b_w.append(need)
    nblk = sum(b_w)

    vals = np.asarray(values, dtype=np.float32)
    n_dma = (nblk + BLK_PER_DMA - 1) // BLK_PER_DMA
    nblk_pad = n_dma * BLK_PER_DMA
    per_core = []
    for c in range(NCORES):
        v_pad = np.zeros((nblk_pad * P, EMBA), dtype=ml_dtypes.bfloat16)
        oh = np.zeros((P, nblk_pad, SEGW), dtype=ml_dtypes.bfloat16)
        gb = 0
        for w in range(W):
            s0 = c * spc + win_lo[w]
            lo = int(seg_start[s0])
            hi = int(seg_start[s0 + win_w[w]])
            r = lo
            for b in range(b_w[w]):
                n = min(P, hi - r)
                if n > 0:
                    v_pad[gb * P : gb * P + n, 0:EMB] = vals[r : r + n]
                    v_pad[gb * P : gb * P + n, EMB] = 1.0
                    loc = (idx[r : r + n] - s0).astype(np.int64)
                    oh[np.arange(n), gb, loc] = 1.0
                r += n
                gb += 1
        # regroup v so each DMA group's data is contiguous per partition:
        # [g, n, p, d] -> [g, p, n, d]; the group-g DMA then reads
        # per-partition-contiguous runs at full HBM bandwidth.
        v_pad = np.ascontiguousarray(
            v_pad.reshape(n_dma, BLK_PER_DMA, P, EMBA).transpose(0, 2, 1, 3)
        ).reshape(n_dma * P, BLK_PER_DMA * EMBA)
        oh = np.ascontiguousarray(oh).reshape(P, nblk_pad * SEGW)
        per_core.append({"v": v_pad, "oh": oh})
    meta = {"W": W, "b_w": tuple(b_w), "win_lo": tuple(win_lo),
            "win_w": tuple(win_w), "nblk": nblk, "spc": spc, "n_dma": n_dma}
    return per_core, meta


# ----------------------------------------------------------------------------
# Bass program (identical for all cores; data differs per core).
# ----------------------------------------------------------------------------
def build_bass(meta, reps=1, ablate=(), debug_taps=False):
    import concourse.bass as bass
    import concourse.bacc as bacc
    import concourse.tile as tile
    from concourse import mybir
    from concourse.bass import broadcast_tensor_aps
    from contextlib import ExitStack

    f32 = mybir.dt.float32
    bf16 = mybir.dt.bfloat16
    Act = mybir.ActivationFunctionType

    W = meta["W"]
    b_w = meta["b_w"]
    win_lo = meta["win_lo"]
    win_w = meta["win_w"]
    nblk = meta["nblk"]
    spc = meta["spc"]
    n_dma = meta["n_dma"]
    n_grp = (spc + GRP - 1) // GRP
    assert spc % GRP == 0 and W * SEGW == spc

    nc = bacc.Bacc(
        "TRN2",
        target_bir_lowering=False,
        debug=False,
        enable_asserts=False,
        num_devices=NCORES,
    )

    v_d = nc.dram_tensor("v", [n_dma * P, BLK_PER_DMA * EMBA], bf16,
                         kind="ExternalInput").ap()
    oh_d = nc.dram_tensor("oh", [P, n_dma * BLK_PER_DMA * SEGW], bf16,
                          kind="ExternalInput").ap()
    gate_d = nc.dram_tensor("gate_rep", [P, EMB], bf16, kind="ExternalInput").ap()
    attn_d = nc.dram_tensor("attn_w", [EMB, EMB], bf16, kind="ExternalInput").ap()
    attnb_d = nc.dram_tensor("attn_b", [P, EMB], f32, kind="ExternalInput").ap()
    ident_d = nc.dram_tensor("ident", [P, P], f32, kind="ExternalInput").ap()
    out_d = nc.dram_tensor("out", [spc, EMB], f32, kind="ExternalOutput").ap()
    dbg_d = None
    if debug_taps:
        dbg_d = nc.dram_tensor("dbg", [P, 256], f32, kind="ExternalOutput").ap()

    with ExitStack() as ctx:
        tc = ctx.enter_context(tile.TileContext(nc))
        const = ctx.enter_context(tc.tile_pool(name="const", bufs=1))
        vpool = ctx.enter_context(tc.tile_pool(name="vpool", bufs=5))
        ohpool = ctx.enter_context(tc.tile_pool(name="ohpool", bufs=5))
        sepool = ctx.enter_context(tc.tile_pool(name="sepool", bufs=5))
        pepool = ctx.enter_context(tc.tile_pool(name="pepool", bufs=5))
        prodpool = ctx.enter_context(tc.tile_pool(name="prodpool", bufs=2))
        scr = ctx.enter_context(tc.tile_pool(name="scr", bufs=1))
        opool = ctx.enter_context(tc.tile_pool(name="opool", bufs=2))
        dram = ctx.enter_context(tc.tile_pool(name="dram", bufs=1, space="DRAM"))
        psum2 = ctx.enter_context(tc.tile_pool(name="psum2", bufs=2, space="PSUM"))
        psum3 = ctx.enter_context(tc.tile_pool(name="psum3", bufs=1, space="PSUM"))
        psumd = ctx.enter_context(tc.tile_pool(name="psumd", bufs=1, space="PSUM"))
        psum1 = ctx.enter_context(tc.tile_pool(name="psum1", bufs=2, space="PSUM"))
        stpool = ctx.enter_context(tc.tile_pool(name="stpool", bufs=2))

        # ---- constants ----
        gate_sb = const.tile([P, 1, EMB], bf16)
        nc.sync.dma_start(out=gate_sb[:, 0, :], in_=gate_d)
        attn0_sb = const.tile([P, EMB], bf16, tag="attn0")
        nc.sync.dma_start(out=attn0_sb, in_=attn_d[0:HALF, :])
        attn1_sb = const.tile([P, EMB], bf16, tag="attn1")
        nc.sync.dma_start(out=attn1_sb, in_=attn_d[HALF:EMB, :])
        attnb_sb = const.tile([P, EMB], f32)
        nc.sync.dma_start(out=attnb_sb, in_=attnb_d)
        ident_sb = const.tile([P, P], f32)
        nc.sync.dma_start(out=ident_sb, in_=ident_d)

        u_stage0 = const.tile([P, n_grp * GRP], bf16, tag="u_stage0")
        u_stage1 = const.tile([P, n_grp * GRP], bf16, tag="u_stage1")
        d_cols = const.tile([SEGW, W], f32, tag="d_cols")
        scratch_act = scr.tile([P, EMB], bf16, tag="scratch_act")

        def one_pass():
            vt_tiles = [None] * n_dma
            pe_tiles = [None] * n_dma

            def ensure_group(g):
                if vt_tiles[g] is not None:
                    return
                nrows = min(BLK_PER_DMA, nblk - g * BLK_PER_DMA)
                vt = vpool.tile([P, BLK_PER_DMA, EMBA], bf16, tag="vt")
                oh_g = ohpool.tile([P, BLK_PER_DMA, SEGW], bf16, tag="oh_g")
                if "dma" not in ablate:
                    nc.sync.dma_start(
                        out=vt.rearrange("p n d -> p (n d)"),
                        in_=v_d[g * P : (g + 1) * P, :],
                    )
                    nc.sync.dma_start(
                        out=oh_g.rearrange("p n s -> p (n s)"),
                        in_=oh_d[:, g * BLK_PER_DMA * SEGW
                                 : (g + 1) * BLK_PER_DMA * SEGW],
                    )
                else:
                    nc.sync.dma_start(out=vt[:, 0, 0:EMB],
                                      in_=v_d[g * P : (g + 1) * P, 0:EMB])
                    nc.sync.dma_start(out=oh_g[:, 0, :],
                                      in_=oh_d[:, 0:SEGW])
                s_g = sepool.tile([P, BLK_PER_DMA], f32, tag="s_g")
                e_g = sepool.tile([P, BLK_PER_DMA], bf16, tag="e_g")
                # gate dot products: one batched packed-bf16 product (DVE
                # tensor_mul runs in the 2x mode) for the whole group, then
                # the per-block free-dim reduces split between the DVE
                # (batched tensor_reduce, 1x) and the ACT engine
                # (Copy+accum_out). Pad blocks reduce zeros -> s=0.
                n_act = (nrows * ACT_FRAC16) // 16
                n_dve = nrows - n_act
                if nrows < BLK_PER_DMA:
                    nc.vector.memset(s_g, 0.0)
                if "amr" not in ablate:
                    # the DVE multiplies + reduces its share; the GPSIMD
                    # multiplies the ACT-reduced share.
                    prod = prodpool.tile([P, BLK_PER_DMA, EMB], bf16,
                                         tag="prod")
                    a_v, a_gate = broadcast_tensor_aps(
                        vt[:, 0:n_dve, 0:EMB], gate_sb[:, :, :])
                    nc.vector.tensor_mul(prod[:, 0:n_dve, :], a_v, a_gate)
                    if n_act:
                        b_v, b_gate = broadcast_tensor_aps(
                            vt[:, n_dve:nrows, 0:EMB], gate_sb[:, :, :])
                        nc.gpsimd.tensor_mul(prod[:, n_dve:nrows, :],
                                             b_v, b_gate)
                    with nc.allow_low_precision("s reduce in one pass"):
                        nc.vector.tensor_reduce(
                            out=s_g[:, 0:n_dve], in_=prod[:, 0:n_dve, :],
                            axis=mybir.AxisListType.X, op=mybir.AluOpType.add)
                    for j in range(n_dve, nrows):
                        nc.scalar.activation(
                            scratch_act, prod[:, j, :], Act.Copy,
                            accum_out=s_g[:, j : j + 1])
                else:
                    nc.vector.memset(s_g, 0.0)
                nc.scalar.activation(e_g, s_g, Act.Exp)
                # P_e = onehot * e  (batched broadcast multiply on the
                # otherwise-idle GPSIMD engine, one per group)
                pe_g = pepool.tile([P, BLK_PER_DMA, SEGW], bf16, tag="pe_g")
                if "tt" not in ablate:
                    e_ap = e_g[:, :]
                    e_3d = bass.AP(e_ap.tensor, e_ap.offset,
                                   [list(d) for d in e_ap.ap] + [[1, 1]])
                    a_oh, a_e = broadcast_tensor_aps(oh_g[:, :, :], e_3d)
                    nc.gpsimd.tensor_mul(pe_g[:, :, :], a_oh, a_e)
                else:
                    nc.vector.tensor_copy(pe_g, oh_g)
                vt_tiles[g] = vt
                pe_tiles[g] = pe_g

            gb = 0
            for w in range(W):
                segw = win_w[w]
                uw = psum2.tile([SEGW, EMBA], f32, tag="uw")
                for b in range(b_w[w]):
                    g, j = divmod(gb, BLK_PER_DMA)
                    ensure_group(g)
                    first = b == 0
                    last = b == b_w[w] - 1
                    if "mm" not in ablate or first or last:
                        nc.tensor.matmul(uw, lhsT=pe_tiles[g][:, j, :],
                                         rhs=vt_tiles[g][:, j, :],
                                         start=first, stop=last)
                    gb += 1
                # ---- window epilogue ----
                off = win_lo[w]
                u_sb = stpool.tile([SEGW, EMBA], f32, tag="u_sb")
                nc.scalar.copy(u_sb, uw)
                t0p = psum3.tile([P, SEGW], f32, tag="t0p")
                nc.tensor.transpose(t0p, u_sb[:, 0:HALF],
                                    ident_sb[0:SEGW, 0:SEGW])
                t1p = psum3.tile([P, SEGW], f32, tag="t1p")
                nc.tensor.transpose(t1p, u_sb[:, HALF:EMB],
                                    ident_sb[0:SEGW, 0:SEGW])
                nc.scalar.copy(u_stage0[:, off : off + segw], t0p[:, 0:segw])
                nc.scalar.copy(u_stage1[:, off : off + segw], t1p[:, 0:segw])
                nc.scalar.copy(d_cols[:, w : w + 1], u_sb[:, EMB:EMBA])

            # ---- D: [seg-in-window, window] -> [group, seg-in-group] rows
            # via DRAM roundtrip, then PE-transpose to per-partition layout.
            d_dram = dram.tile([SEGW, W], f32, tag="d_dram")
            nc.sync.dma_start(out=d_dram, in_=d_cols)
            d_rows = const.tile([P, GRP], f32, tag="d_rows")
            nc.vector.memset(d_rows, 0.0)
            nc.sync.dma_start(
                out=d_rows[0:n_grp, :].rearrange("g (a r) -> g a r", r=SEGW),
                in_=d_dram.rearrange("r (g a) -> g a r", g=n_grp),
            )
            dT = psumd.tile([P, P], f32, tag="dT")
            nc.tensor.transpose(dT, d_rows, ident_sb)
            d_sq = const.tile([P, n_grp], f32, tag="d_sq")
            nc.vector.tensor_copy(d_sq, dT[:, 0:n_grp])
            d_cl = const.tile([P, n_grp], f32, tag="d_cl")
            nc.vector.tensor_scalar_max(d_cl, d_sq, 1e-30)
            rec = const.tile([P, n_grp], f32, tag="rec")
            nc.vector.reciprocal(rec, d_cl)
            if debug_taps:
                nc.sync.dma_start(out=dbg_d[:, 0:n_grp], in_=d_sq)
                nc.sync.dma_start(out=dbg_d[:, 4:4+n_grp], in_=rec)
                nc.sync.dma_start(out=dbg_d[0:SEGW, 8:8+W], in_=d_cols)
                nc.sync.dma_start(out=dbg_d[44:44+n_grp, 128:256],
                                  in_=d_rows[0:n_grp, :])
                nc.sync.dma_start(out=dbg_d[48:80, 64:128],
                                  in_=u_stage0[0:32, 0:64])

            # ---- final: Z = U @ attn_w + D * attn_b, out = Z / D ----
            for g in range(n_grp):
                lo = g * GRP
                z = psum1.tile([GRP, EMB], f32, tag="z")
                nc.tensor.matmul(z, lhsT=u_stage0[:, lo : lo + GRP], rhs=attn0_sb,
                                 start=True, stop=False)
                nc.tensor.matmul(z, lhsT=u_stage1[:, lo : lo + GRP], rhs=attn1_sb,
                                 start=False, stop=True)
                o_sb = opool.tile([GRP, EMB], f32, tag="o_sb")
                nc.scalar.activation(o_sb, z, Act.Copy, scale=rec[:, g : g + 1])
                nc.vector.tensor_add(o_sb, o_sb, attnb_sb)
                nc.sync.dma_start(out=out_d[lo : lo + GRP, :], in_=o_sb)

        for _rep in range(reps):
            one_pass()

    nc.compile()
    return nc


def _get_program(meta, reps=1):
    key = (meta["W"], meta["b_w"], meta["win_lo"], meta["win_w"],
           meta["spc"], reps)
    if key not in _CACHE:
        _CACHE[key] = build_bass(meta, reps=reps)
    return _CACHE[key]


def make_const_inputs(gate_w, attn_w, attn_b):
    gate_rep = np.ascontiguousarray(
        np.broadcast_to(np.asarray(gate_w, np.float32).reshape(1, EMB),
                        (P, EMB))).astype(ml_dtypes.bfloat16)
    return {
        "gate_rep": gate_rep,
        "attn_w": np.asarray(attn_w, np.float32).astype(ml_dtypes.bfloat16),
        "attn_b": np.ascontiguousarray(np.broadcast_to(
            np.asarray(attn_b, np.float32).reshape(1, EMB), (P, EMB))),
        "ident": np.eye(P, dtype=np.float32),
    }


def build_in_maps(values, indices, num_graphs, gate_w, attn_w, attn_b):
    G = int(num_graphs)
    per_core, meta = prepare_host(values, indices, G)
    consts = make_const_inputs(gate_w, attn_w, attn_b)
    in_maps = [{**consts, "v": pc["v"], "oh": pc["oh"]} for pc in per_core]
    return in_maps, meta


# ----------------------------------------------------------------------------
# Public entry point.
# ----------------------------------------------------------------------------
def kernel(values, indices, num_graphs, gate_w, gate_b, attn_w, attn_b):
    from concourse.bass_utils import run_bass_kernel_spmd

    in_maps, meta = build_in_maps(values, indices, num_graphs,
                                  gate_w, attn_w, attn_b)
    nc = _get_program(meta)
    res = run_bass_kernel_spmd(nc, in_maps, core_ids=list(range(NCORES)))
    out = np.concatenate([res.results[c]["out"] for c in range(NCORES)], axis=0)
    return out[: int(num_graphs)]


# revision 71
# speedup vs baseline: 1.0072x; 1.0072x over previous
"""Trainium2 Bass kernel for AttentionalAggregation (segment softmax-weighted sum).

reference math:
    s = values @ gate_w + gate_b            # [N,1]
    w = segment_softmax(s, indices)         # [N,1]
    out = segment_sum(w * (values @ attn_w + attn_b))   # [G,EMB]

Algebraic restructuring (exact up to fp rounding):
  softmax weights per segment sum to 1, so
      out[g] = (U[g]/D[g]) @ attn_w + attn_b
  with U[g] = sum_{i in g} e_i * values_i, D[g] = sum_{i in g} e_i,
  e_i = exp(values_i . gate_w).  gate_b and the per-segment max shift
  cancel in the U/D ratio (|s| <= ~4 for this data, exp can't overflow).

Sharding: indices are sorted, so each of the 8 cores owns G/8 contiguous
segments and their (contiguous) nodes. No collectives.

This version streams `values` in bf16 (the 2e-2 relative-error budget has
plenty of room: bf16 rounding contributes ~3e-3), which halves HBM traffic
and runs the PE at 1 cycle/row instead of fp32's 4.  Each value row is
augmented with a trailing 1.0 column so ONE matmul per 128-node block
yields both U (cols 0:256) and D (col 256).  The per-node one-hot segment
masks are precomputed on the host (they are a re-encoding of `indices`)
and DMA'd in bf16 (SEGW/EMB ~ 6% extra bytes), so the device-side
per-group (16-block) work is:
  - DVE tensor_mul (packed bf16, 2x mode): prod = v * gate, batched
  - the per-block free-dim reduces s[p] = sum_j prod[p,j] split between
    the DVE (batched tensor_reduce, 11/16) and ACT (Copy+accum_out, 5/16)
    -- measured on HW these balance at ~5us/group each
  - ACT exp (one batched op), GPSIMD broadcast-multiply P_e = onehot * e
  - PE matmul per block (accumulated over the window's blocks in PSUM):
        uw[0:SEGW, 0:257] += P_e.T @ [v | 1]
The window epilogue transposes
uw[.., 0:256] into per-core staging tiles and copies the D column; the
final phase computes Z = U @ attn_w + D*attn_b with 3 matmuls per
128-segment group and scales by 1/D via ACT per-partition scale (D is
rearranged into per-partition layout by a tiny DRAM round-trip).

Everything is static: no sequencer registers, no dynamic access patterns.
"""

import numpy as np
import ml_dtypes

P = 128
EMB = 256
EMBA = EMB + 1      # v rows augmented with a ones column
HALF = 128
SEGW = 16           # segments per window == one-hot width
ACT_FRAC16 = 5      # blocks per 16 whose product runs on GPSIMD
ACT_EXTRA = 1       # extra DVE-multiplied blocks whose reduce runs on ACT
NCORES = 8
BLK_PER_DMA = 16    # blocks per DMA group
GRP = 128           # segments per final-matmul group

_CACHE = {}


# ----------------------------------------------------------------------------
# Host-side preparation: shard + pad nodes into (core, window, block) layout.
# ----------------------------------------------------------------------------
def prepare_host(values, indices, G):
    idx = np.ascontiguousarray(np.asarray(indices).astype(np.int64))
    counts = np.bincount(idx, minlength=G)
    seg_start = np.zeros(G + 1, dtype=np.int64)
    np.cumsum(counts, out=seg_start[1:])

    assert G % NCORES == 0
    spc = G // NCORES                      # segments per core
    win_lo = list(range(0, spc, SEGW))     # window seg offsets within a core
    win_w = [min(SEGW, spc - lo) for lo in win_lo]
    W = len(win_lo)

    # blocks per window index = max over cores (SPMD: one program, 8 cores)
    b_w = []
    for w in range(W):
        need = 1
        for c in range(NCORES):
            s0 = c * spc + win_lo[w]
            n = int(seg_start[s0 + win_w[w]] - seg_start[s0])
            need = max(need, (n + P - 1) // P)
        b_w.append(need)
    nblk = sum(b_w)

    vals = np.asarray(values, dtype=np.float32)
    n_dma = (nblk + BLK_PER_DMA - 1) // BLK_PER_DMA
    nblk_pad = n_dma * BLK_PER_DMA
    per_core = []
    for c in range(NCORES):
        v_pad = np.zeros((nblk_pad * P, EMBA), dtype=ml_dtypes.bfloat16)
        oh = np.zeros((P, nblk_pad, SEGW), dtype=ml_dtypes.bfloat16)
        gb = 0
        for w in range(W):
            s0 = c * spc + win_lo[w]
            lo = int(seg_start[s0])
            hi = int(seg_start[s0 + win_w[w]])
            r = lo
            for b in range(b_w[w]):
                n = min(P, hi - r)
                if n > 0:
                    v_pad[gb * P : gb * P + n, 0:EMB] = vals[r : r + n]
                    v_pad[gb * P : gb * P + n, EMB] = 1.0
                    loc = (idx[r : r + n] - s0).astype(np.int64)
                    oh[np.arange(n), gb, loc] = 1.0
                r += n
                gb += 1
        # regroup v so each DMA group's data is contiguous per partition:
        # [g, n, p, d] -> [g, p, n, d]; the group-g DMA then reads
        # per-partition-contiguous runs at full HBM bandwidth.
        v_pad = np.ascontiguousarray(
            v_pad.reshape(n_dma, BLK_PER_DMA, P, EMBA).transpose(0, 2, 1, 3)
        ).reshape(n_dma * P, BLK_PER_DMA * EMBA)
        oh = np.ascontiguousarray(oh).reshape(P, nblk_pad * SEGW)
        per_core.append({"v": v_pad, "oh": oh})
    meta = {"W": W, "b_w": tuple(b_w), "win_lo": tuple(win_lo),
            "win_w": tuple(win_w), "nblk": nblk, "spc": spc, "n_dma": n_dma}
    return per_core, meta


# ----------------------------------------------------------------------------
# Bass program (identical for all cores; data differs per core).
# ----------------------------------------------------------------------------
def build_bass(meta, reps=1, ablate=(), debug_taps=False):
    import concourse.bass as bass
    import concourse.bacc as bacc
    import concourse.tile as tile
    from concourse import mybir
    from concourse.bass import broadcast_tensor_aps
    from contextlib import ExitStack

    f32 = mybir.dt.float32
    bf16 = mybir.dt.bfloat16
    Act = mybir.ActivationFunctionType

    W = meta["W"]
    b_w = meta["b_w"]
    win_lo = meta["win_lo"]
    win_w = meta["win_w"]
    nblk = meta["nblk"]
    spc = meta["spc"]
    n_dma = meta["n_dma"]
    n_grp = (spc + GRP - 1) // GRP
    assert spc % GRP == 0 and W * SEGW == spc

    nc = bacc.Bacc(
        "TRN2",
        target_bir_lowering=False,
        debug=False,
        enable_asserts=False,
        num_devices=NCORES,
    )

    v_d = nc.dram_tensor("v", [n_dma * P, BLK_PER_DMA * EMBA], bf16,
                         kind="ExternalInput").ap()
    oh_d = nc.dram_tensor("oh", [P, n_dma * BLK_PER_DMA * SEGW], bf16,
                          kind="ExternalInput").ap()
    gate_d = nc.dram_tensor("gate_rep", [P, EMB], bf16, kind="ExternalInput").ap()
    attn_d = nc.dram_tensor("attn_w", [EMB, EMB], bf16, kind="ExternalInput").ap()
    attnb_d = nc.dram_tensor("attn_b", [P, EMB], f32, kind="ExternalInput").ap()
    ident_d = nc.dram_tensor("ident", [P, P], f32, kind="ExternalInput").ap()
    out_d = nc.dram_tensor("out", [spc, EMB], f32, kind="ExternalOutput").ap()
    dbg_d = None
    if debug_taps:
        dbg_d = nc.dram_tensor("dbg", [P, 256], f32, kind="ExternalOutput").ap()

    with ExitStack() as ctx:
        tc = ctx.enter_context(tile.TileContext(nc))
        const = ctx.enter_context(tc.tile_pool(name="const", bufs=1))
        vpool = ctx.enter_context(tc.tile_pool(name="vpool", bufs=5))
        ohpool = ctx.enter_context(tc.tile_pool(name="ohpool", bufs=5))
        sepool = ctx.enter_context(tc.tile_pool(name="sepool", bufs=5))
        pepool = ctx.enter_context(tc.tile_pool(name="pepool", bufs=5))
        prodpool = ctx.enter_context(tc.tile_pool(name="prodpool", bufs=2))
        scr = ctx.enter_context(tc.tile_pool(name="scr", bufs=1))
        opool = ctx.enter_context(tc.tile_pool(name="opool", bufs=2))
        dram = ctx.enter_context(tc.tile_pool(name="dram", bufs=1, space="DRAM"))
        psum2 = ctx.enter_context(tc.tile_pool(name="psum2", bufs=2, space="PSUM"))
        psum3 = ctx.enter_context(tc.tile_pool(name="psum3", bufs=1, space="PSUM"))
        psumd = ctx.enter_context(tc.tile_pool(name="psumd", bufs=1, space="PSUM"))
        psum1 = ctx.enter_context(tc.tile_pool(name="psum1", bufs=2, space="PSUM"))
        stpool = ctx.enter_context(tc.tile_pool(name="stpool", bufs=2))

        # ---- constants ----
        gate_sb = const.tile([P, 1, EMB], bf16)
        nc.sync.dma_start(out=gate_sb[:, 0, :], in_=gate_d)
        attn0_sb = const.tile([P, EMB], bf16, tag="attn0")
        nc.sync.dma_start(out=attn0_sb, in_=attn_d[0:HALF, :])
        attn1_sb = const.tile([P, EMB], bf16, tag="attn1")
        nc.sync.dma_start(out=attn1_sb, in_=attn_d[HALF:EMB, :])
        attnb_sb = const.tile([P, EMB], f32)
        nc.sync.dma_start(out=attnb_sb, in_=attnb_d)
        ident_sb = const.tile([P, P], f32)
        nc.sync.dma_start(out=ident_sb, in_=ident_d)

        u_stage0 = const.tile([P, n_grp * GRP], bf16, tag="u_stage0")
        u_stage1 = const.tile([P, n_grp * GRP], bf16, tag="u_stage1")
        d_cols = const.tile([SEGW, W], f32, tag="d_cols")
        scratch_act = scr.tile([P, EMB], bf16, tag="scratch_act")

        def one_pass():
            vt_tiles = [None] * n_dma
            pe_tiles = [None] * n_dma

            def ensure_group(g):
                if vt_tiles[g] is not None:
                    return
                nrows = min(BLK_PER_DMA, nblk - g * BLK_PER_DMA)
                vt = vpool.tile([P, BLK_PER_DMA, EMBA], bf16, tag="vt")
                oh_g = ohpool.tile([P, BLK_PER_DMA, SEGW], bf16, tag="oh_g")
                if "dma" not in ablate:
                    nc.sync.dma_start(
                        out=vt.rearrange("p n d -> p (n d)"),
                        in_=v_d[g * P : (g + 1) * P, :],
                    )
                    nc.sync.dma_start(
                        out=oh_g.rearrange("p n s -> p (n s)"),
                        in_=oh_d[:, g * BLK_PER_DMA * SEGW
                                 : (g + 1) * BLK_PER_DMA * SEGW],
                    )
                else:
                    nc.sync.dma_start(out=vt[:, 0, 0:EMB],
                                      in_=v_d[g * P : (g + 1) * P, 0:EMB])
                    nc.sync.dma_start(out=oh_g[:, 0, :],
                                      in_=oh_d[:, 0:SEGW])
                s_g = sepool.tile([P, BLK_PER_DMA], f32, tag="s_g")
                e_g = sepool.tile([P, BLK_PER_DMA], bf16, tag="e_g")
                # gate dot products: one batched packed-bf16 product (DVE
                # tensor_mul runs in the 2x mode) for the whole group, then
                # the per-block free-dim reduces split between the DVE
                # (batched tensor_reduce, 1x) and the ACT engine
                # (Copy+accum_out). Pad blocks reduce zeros -> s=0.
                n_act = (nrows * ACT_FRAC16) // 16
                n_dve = nrows - n_act
                if nrows < BLK_PER_DMA:
                    nc.vector.memset(s_g, 0.0)
                if "amr" not in ablate:
                    # the DVE multiplies + reduces its share; the GPSIMD
                    # multiplies the ACT-reduced share.
                    prod = prodpool.tile([P, BLK_PER_DMA, EMB], bf16,
                                         tag="prod")
                    a_v, a_gate = broadcast_tensor_aps(
                        vt[:, 0:n_dve, 0:EMB], gate_sb[:, :, :])
                    nc.vector.tensor_mul(prod[:, 0:n_dve, :], a_v, a_gate)
                    if n_act:
                        b_v, b_gate = broadcast_tensor_aps(
                            vt[:, n_dve:nrows, 0:EMB], gate_sb[:, :, :])
                        nc.gpsimd.tensor_mul(prod[:, n_dve:nrows, :],
                                             b_v, b_gate)
                    n_tr = max(0, n_dve - ACT_EXTRA)
                    with nc.allow_low_precision("s reduce in one pass"):
                        nc.vector.tensor_reduce(
                            out=s_g[:, 0:n_tr], in_=prod[:, 0:n_tr, :],
                            axis=mybir.AxisListType.X, op=mybir.AluOpType.add)
                    for j in range(n_tr, nrows):
                        nc.scalar.activation(
                            scratch_act, prod[:, j, :], Act.Copy,
                            accum_out=s_g[:, j : j + 1])
                else:
                    nc.vector.memset(s_g, 0.0)
                nc.scalar.activation(e_g, s_g, Act.Exp)
                # P_e = onehot * e  (batched broadcast multiply on the
                # otherwise-idle GPSIMD engine, one per group)
                pe_g = pepool.tile([P, BLK_PER_DMA, SEGW], bf16, tag="pe_g")
                if "tt" not in ablate:
                    e_ap = e_g[:, :]
                    e_3d = bass.AP(e_ap.tensor, e_ap.offset,
                                   [list(d) for d in e_ap.ap] + [[1, 1]])
                    a_oh, a_e = broadcast_tensor_aps(oh_g[:, :, :], e_3d)
                    nc.gpsimd.tensor_mul(pe_g[:, :, :], a_oh, a_e)
                else:
                    nc.vector.tensor_copy(pe_g, oh_g)
                vt_tiles[g] = vt
                pe_tiles[g] = pe_g

            gb = 0
            for w in range(W):
                segw = win_w[w]
                uw = psum2.tile([SEGW, EMBA], f32, tag="uw")
                for b in range(b_w[w]):
                    g, j = divmod(gb, BLK_PER_DMA)
                    ensure_group(g)
                    first = b == 0
                    last = b == b_w[w] - 1
                    if "mm" not in ablate or first or last:
                        nc.tensor.matmul(uw, lhsT=pe_tiles[g][:, j, :],
                                         rhs=vt_tiles[g][:, j, :],
                                         start=first, stop=last)
                    gb += 1
                # ---- window epilogue ----
                off = win_lo[w]
                u_sb = stpool.tile([SEGW, EMBA], f32, tag="u_sb")
                nc.scalar.copy(u_sb, uw)
                t0p = psum3.tile([P, SEGW], f32, tag="t0p")
                nc.tensor.transpose(t0p, u_sb[:, 0:HALF],
                                    ident_sb[0:SEGW, 0:SEGW])
                t1p = psum3.tile([P, SEGW], f32, tag="t1p")
                nc.tensor.transpose(t1p, u_sb[:, HALF:EMB],
                                    ident_sb[0:SEGW, 0:SEGW])
                nc.scalar.copy(u_stage0[:, off : off + segw], t0p[:, 0:segw])
                nc.scalar.copy(u_stage1[:, off : off + segw], t1p[:, 0:segw])
                nc.scalar.copy(d_cols[:, w : w + 1], u_sb[:, EMB:EMBA])

            # ---- D: [seg-in-window, window] -> [group, seg-in-group] rows
            # via DRAM roundtrip, then PE-transpose to per-partition layout.
            d_dram = dram.tile([SEGW, W], f32, tag="d_dram")
            nc.sync.dma_start(out=d_dram, in_=d_cols)
            d_rows = const.tile([P, GRP], f32, tag="d_rows")
            nc.vector.memset(d_rows, 0.0)
            nc.sync.dma_start(
                out=d_rows[0:n_grp, :].rearrange("g (a r) -> g a r", r=SEGW),
                in_=d_dram.rearrange("r (g a) -> g a r", g=n_grp),
            )
            dT = psumd.tile([P, P], f32, tag="dT")
            nc.tensor.transpose(dT, d_rows, ident_sb)
            d_sq = const.tile([P, n_grp], f32, tag="d_sq")
            nc.vector.tensor_copy(d_sq, dT[:, 0:n_grp])
            d_cl = const.tile([P, n_grp], f32, tag="d_cl")
            nc.vector.tensor_scalar_max(d_cl, d_sq, 1e-30)
            rec = const.tile([P, n_grp], f32, tag="rec")
            nc.vector.reciprocal(rec, d_cl)
            if debug_taps:
                nc.sync.dma_start(out=dbg_d[:, 0:n_grp], in_=d_sq)
                nc.sync.dma_start(out=dbg_d[:, 4:4+n_grp], in_=rec)
                nc.sync.dma_start(out=dbg_d[0:SEGW, 8:8+W], in_=d_cols)
                nc.sync.dma_start(out=dbg_d[44:44+n_grp, 128:256],
                                  in_=d_rows[0:n_grp, :])
                nc.sync.dma_start(out=dbg_d[48:80, 64:128],
                                  in_=u_stage0[0:32, 0:64])

            # ---- final: Z = U @ attn_w + D * attn_b, out = Z / D ----
            for g in range(n_grp):
                lo = g * GRP
                z = psum1.tile([GRP, EMB], f32, tag="z")
                nc.tensor.matmul(z, lhsT=u_stage0[:, lo : lo + GRP], rhs=attn0_sb,
                                 start=True, stop=False)
                nc.tensor.matmul(z, lhsT=u_stage1[:, lo : lo + GRP], rhs=attn1_sb,
                                 start=False, stop=True)
                o_sb = opool.tile([GRP, EMB], f32, tag="o_sb")
                nc.scalar.activation(o_sb, z, Act.Copy, scale=rec[:, g : g + 1])
                nc.vector.tensor_add(o_sb, o_sb, attnb_sb)
                nc.sync.dma_start(out=out_d[lo : lo + GRP, :], in_=o_sb)

        for _rep in range(reps):
            one_pass()

    nc.compile()
    return nc


def _get_program(meta, reps=1):
    key = (meta["W"], meta["b_w"], meta["win_lo"], meta["win_w"],
           meta["spc"], reps)
    if key not in _CACHE:
        _CACHE[key] = build_bass(meta, reps=reps)
    return _CACHE[key]


def make_const_inputs(gate_w, attn_w, attn_b):
    gate_rep = np.ascontiguousarray(
        np.broadcast_to(np.asarray(gate_w, np.float32).reshape(1, EMB),
                        (P, EMB))).astype(ml_dtypes.bfloat16)
    return {
        "gate_rep": gate_rep,
        "attn_w": np.asarray(attn_w, np.float32).astype(ml_dtypes.bfloat16),
        "attn_b": np.ascontiguousarray(np.broadcast_to(
            np.asarray(attn_b, np.float32).reshape(1, EMB), (P, EMB))),
        "ident": np.eye(P, dtype=np.float32),
    }


def build_in_maps(values, indices, num_graphs, gate_w, attn_w, attn_b):
    G = int(num_graphs)
    per_core, meta = prepare_host(values, indices, G)
    consts = make_const_inputs(gate_w, attn_w, attn_b)
    in_maps = [{**consts, "v": pc["v"], "oh": pc["oh"]} for pc in per_core]
    return in_maps, meta


# ----------------------------------------------------------------------------
# Public entry point.
# ----------------------------------------------------------------------------
def kernel(values, indices, num_graphs, gate_w, gate_b, attn_w, attn_b):
    from concourse.bass_utils import run_bass_kernel_spmd

    in_maps, meta = build_in_maps(values, indices, num_graphs,
                                  gate_w, attn_w, attn_b)
    nc = _get_program(meta)
    res = run_bass_kernel_spmd(nc, in_maps, core_ids=list(range(NCORES)))
    out = np.concatenate([res.results[c]["out"] for c in range(NCORES)], axis=0)
    return out[: int(num_graphs)]


# revision 72
# speedup vs baseline: 1.8239x; 1.8109x over previous
"""Trainium2 Bass kernel for AttentionalAggregation (segment softmax-weighted sum).

reference math:
    s = values @ gate_w + gate_b            # [N,1]
    w = segment_softmax(s, indices)         # [N,1]
    out = segment_sum(w * (values @ attn_w + attn_b))   # [G,EMB]

Algebraic restructuring (exact up to fp rounding):
  softmax weights per segment sum to 1, so
      out[g] = (U[g]/D[g]) @ attn_w + attn_b
  with U[g] = sum_{i in g} e_i * values_i, D[g] = sum_{i in g} e_i,
  e_i = exp(values_i . gate_w).  gate_b and the per-segment max shift
  cancel in the U/D ratio (|s| <= ~4 for this data, exp can't overflow).

Sharding: indices are sorted, so each of the 8 cores owns G/8 contiguous
segments and their (contiguous) nodes. No collectives.

This version streams `values` in bf16 (the 2e-2 relative-error budget has
plenty of room: bf16 rounding contributes ~3e-3), which halves HBM traffic
and runs the PE at 1 cycle/row instead of fp32's 4.  Each value row is
augmented with a trailing 1.0 column so ONE matmul per 128-node block
yields both U (cols 0:256) and D (col 256).  The per-node one-hot segment
masks are precomputed on the host (they are a re-encoding of `indices`)
and DMA'd in bf16 (SEGW/EMB ~ 6% extra bytes), so the device-side
per-group (16-block) work is:
  - DVE tensor_mul (packed bf16, 2x mode): prod = v * gate, batched
  - the per-block free-dim reduces s[p] = sum_j prod[p,j] split between
    the DVE (batched tensor_reduce, 11/16) and ACT (Copy+accum_out, 5/16)
    -- measured on HW these balance at ~5us/group each
  - ACT exp (one batched op), GPSIMD broadcast-multiply P_e = onehot * e
  - PE matmul per block (accumulated over the window's blocks in PSUM):
        uw[0:SEGW, 0:257] += P_e.T @ [v | 1]
The window epilogue transposes
uw[.., 0:256] into per-core staging tiles and copies the D column; the
final phase computes Z = U @ attn_w + D*attn_b with 3 matmuls per
128-segment group and scales by 1/D via ACT per-partition scale (D is
rearranged into per-partition layout by a tiny DRAM round-trip).

Everything is static: no sequencer registers, no dynamic access patterns.
"""

import numpy as np
import ml_dtypes

P = 128
EMB = 256
EMBA = EMB + 1      # v rows augmented with a ones column
HALF = 128
SEGW = 16           # segments per window == one-hot width
ACT_FRAC16 = 5      # blocks per 16 whose gate-dot reduce runs on ACT
NCORES = 8
BLK_PER_DMA = 16    # blocks per DMA group
GRP = 128           # segments per final-matmul group

_CACHE = {}


# ----------------------------------------------------------------------------
# Host-side preparation: shard + pad nodes into (core, window, block) layout.
# ----------------------------------------------------------------------------
def prepare_host(values, indices, G):
    idx = np.ascontiguousarray(np.asarray(indices).astype(np.int64))
    counts = np.bincount(idx, minlength=G)
    seg_start = np.zeros(G + 1, dtype=np.int64)
    np.cumsum(counts, out=seg_start[1:])

    assert G % NCORES == 0
    spc = G // NCORES                      # segments per core
    win_lo = list(range(0, spc, SEGW))     # window seg offsets within a core
    win_w = [min(SEGW, spc - lo) for lo in win_lo]
    W = len(win_lo)

    # blocks per window index = max over cores (SPMD: one program, 8 cores)
    b_w = []
    for w in range(W):
        need = 1
        for c in range(NCORES):
            s0 = c * spc + win_lo[w]
            n = int(seg_start[s0 + win_w[w]] - seg_start[s0])
            need = max(need, (n + P - 1) // P)
        b_w.append(need)
    nblk = sum(b_w)

    vals = np.asarray(values, dtype=np.float32)
    n_dma = (nblk + BLK_PER_DMA - 1) // BLK_PER_DMA
    nblk_pad = n_dma * BLK_PER_DMA
    per_core = []
    for c in range(NCORES):
        v_pad = np.zeros((nblk_pad * P, EMBA), dtype=ml_dtypes.bfloat16)
        oh = np.zeros((P, nblk_pad, SEGW), dtype=ml_dtypes.bfloat16)
        gb = 0
        for w in range(W):
            s0 = c * spc + win_lo[w]
            lo = int(seg_start[s0])
            hi = int(seg_start[s0 + win_w[w]])
            r = lo
            for b in range(b_w[w]):
                n = min(P, hi - r)
                if n > 0:
                    v_pad[gb * P : gb * P + n, 0:EMB] = vals[r : r + n]
                    v_pad[gb * P : gb * P + n, EMB] = 1.0
                    loc = (idx[r : r + n] - s0).astype(np.int64)
                    oh[np.arange(n), gb, loc] = 1.0
                r += n
                gb += 1
        # regroup v so each DMA group's data is contiguous per partition:
        # [g, n, p, d] -> [g, p, n, d]; the group-g DMA then reads
        # per-partition-contiguous runs at full HBM bandwidth.
        v_pad = np.ascontiguousarray(
            v_pad.reshape(n_dma, BLK_PER_DMA, P, EMBA).transpose(0, 2, 1, 3)
        ).reshape(n_dma * P, BLK_PER_DMA * EMBA)
        oh = np.ascontiguousarray(oh).reshape(P, nblk_pad * SEGW)
        per_core.append({"v": v_pad, "oh": oh})
    meta = {"W": W, "b_w": tuple(b_w), "win_lo": tuple(win_lo),
            "win_w": tuple(win_w), "nblk": nblk, "spc": spc, "n_dma": n_dma}
    return per_core, meta


# ----------------------------------------------------------------------------
# Bass program (identical for all cores; data differs per core).
# ----------------------------------------------------------------------------
def build_bass(meta, reps=1, ablate=(), debug_taps=False):
    import concourse.bass as bass
    import concourse.bacc as bacc
    import concourse.tile as tile
    from concourse import mybir
    from concourse.bass import broadcast_tensor_aps
    from contextlib import ExitStack

    f32 = mybir.dt.float32
    bf16 = mybir.dt.bfloat16
    Act = mybir.ActivationFunctionType

    W = meta["W"]
    b_w = meta["b_w"]
    win_lo = meta["win_lo"]
    win_w = meta["win_w"]
    nblk = meta["nblk"]
    spc = meta["spc"]
    n_dma = meta["n_dma"]
    n_grp = (spc + GRP - 1) // GRP
    assert spc % GRP == 0 and W * SEGW == spc

    nc = bacc.Bacc(
        "TRN2",
        target_bir_lowering=False,
        debug=False,
        enable_asserts=False,
        num_devices=NCORES,
    )

    v_d = nc.dram_tensor("v", [n_dma * P, BLK_PER_DMA * EMBA], bf16,
                         kind="ExternalInput").ap()
    oh_d = nc.dram_tensor("oh", [P, n_dma * BLK_PER_DMA * SEGW], bf16,
                          kind="ExternalInput").ap()
    gate_d = nc.dram_tensor("gate_rep", [P, EMB], bf16, kind="ExternalInput").ap()
    attn_d = nc.dram_tensor("attn_w", [EMB, EMB], bf16, kind="ExternalInput").ap()
    attnb_d = nc.dram_tensor("attn_b", [P, EMB], f32, kind="ExternalInput").ap()
    ident_d = nc.dram_tensor("ident", [P, P], f32, kind="ExternalInput").ap()
    out_d = nc.dram_tensor("out", [spc, EMB], f32, kind="ExternalOutput").ap()
    dbg_d = None
    if debug_taps:
        dbg_d = nc.dram_tensor("dbg", [P, 256], f32, kind="ExternalOutput").ap()

    with ExitStack() as ctx:
        tc = ctx.enter_context(tile.TileContext(nc))
        const = ctx.enter_context(tc.tile_pool(name="const", bufs=1))
        vpool = ctx.enter_context(tc.tile_pool(name="vpool", bufs=5))
        ohpool = ctx.enter_context(tc.tile_pool(name="ohpool", bufs=5))
        sepool = ctx.enter_context(tc.tile_pool(name="sepool", bufs=5))
        pepool = ctx.enter_context(tc.tile_pool(name="pepool", bufs=5))
        prodpool = ctx.enter_context(tc.tile_pool(name="prodpool", bufs=2))
        scr = ctx.enter_context(tc.tile_pool(name="scr", bufs=1))
        opool = ctx.enter_context(tc.tile_pool(name="opool", bufs=2))
        dram = ctx.enter_context(tc.tile_pool(name="dram", bufs=1, space="DRAM"))
        psum2 = ctx.enter_context(tc.tile_pool(name="psum2", bufs=2, space="PSUM"))
        psum3 = ctx.enter_context(tc.tile_pool(name="psum3", bufs=1, space="PSUM"))
        psumd = ctx.enter_context(tc.tile_pool(name="psumd", bufs=1, space="PSUM"))
        psum1 = ctx.enter_context(tc.tile_pool(name="psum1", bufs=2, space="PSUM"))
        stpool = ctx.enter_context(tc.tile_pool(name="stpool", bufs=2))

        # ---- constants ----
        gate_sb = const.tile([P, 1, EMB], bf16)
        nc.sync.dma_start(out=gate_sb[:, 0, :], in_=gate_d)
        attn0_sb = const.tile([P, EMB], bf16, tag="attn0")
        nc.sync.dma_start(out=attn0_sb, in_=attn_d[0:HALF, :])
        attn1_sb = const.tile([P, EMB], bf16, tag="attn1")
        nc.sync.dma_start(out=attn1_sb, in_=attn_d[HALF:EMB, :])
        attnb_sb = const.tile([P, EMB], f32)
        nc.sync.dma_start(out=attnb_sb, in_=attnb_d)
        ident_sb = const.tile([P, P], f32)
        nc.sync.dma_start(out=ident_sb, in_=ident_d)

        u_stage0 = const.tile([P, n_grp * GRP], bf16, tag="u_stage0")
        u_stage1 = const.tile([P, n_grp * GRP], bf16, tag="u_stage1")
        d_cols = const.tile([SEGW, W], f32, tag="d_cols")
        scratch_act = scr.tile([P, EMB], bf16, tag="scratch_act")

        def one_pass():
            vt_tiles = [None] * n_dma
            pe_tiles = [None] * n_dma

            def ensure_group(g):
                if vt_tiles[g] is not None:
                    return
                nrows = min(BLK_PER_DMA, nblk - g * BLK_PER_DMA)
                vt = vpool.tile([P, BLK_PER_DMA, EMBA], bf16, tag="vt")
                oh_g = ohpool.tile([P, BLK_PER_DMA, SEGW], bf16, tag="oh_g")
                if "dma" not in ablate:
                    nc.sync.dma_start(
                        out=vt.rearrange("p n d -> p (n d)"),
                        in_=v_d[g * P : (g + 1) * P, :],
                    )
                    nc.sync.dma_start(
                        out=oh_g.rearrange("p n s -> p (n s)"),
                        in_=oh_d[:, g * BLK_PER_DMA * SEGW
                                 : (g + 1) * BLK_PER_DMA * SEGW],
                    )
                else:
                    nc.sync.dma_start(out=vt[:, 0, 0:EMB],
                                      in_=v_d[g * P : (g + 1) * P, 0:EMB])
                    nc.sync.dma_start(out=oh_g[:, 0, :],
                                      in_=oh_d[:, 0:SEGW])
                s_g = sepool.tile([P, BLK_PER_DMA], f32, tag="s_g")
                e_g = sepool.tile([P, BLK_PER_DMA], bf16, tag="e_g")
                # gate dot products: one batched packed-bf16 product (DVE
                # tensor_mul runs in the 2x mode) for the whole group, then
                # the per-block free-dim reduces split between the DVE
                # (batched tensor_reduce, 1x) and the ACT engine
                # (Copy+accum_out). Pad blocks reduce zeros -> s=0.
                n_act = (nrows * ACT_FRAC16) // 16
                n_dve = nrows - n_act
                if nrows < BLK_PER_DMA:
                    nc.vector.memset(s_g, 0.0)
                if "amr" not in ablate:
                    # the DVE multiplies + reduces its share; the GPSIMD
                    # multiplies the ACT-reduced share.
                    prod = prodpool.tile([P, BLK_PER_DMA, EMB], bf16,
                                         tag="prod")
                    a_v, a_gate = broadcast_tensor_aps(
                        vt[:, 0:n_dve, 0:EMB], gate_sb[:, :, :])
                    nc.vector.tensor_mul(prod[:, 0:n_dve, :], a_v, a_gate)
                    if n_act:
                        b_v, b_gate = broadcast_tensor_aps(
                            vt[:, n_dve:nrows, 0:EMB], gate_sb[:, :, :])
                        nc.gpsimd.tensor_mul(prod[:, n_dve:nrows, :],
                                             b_v, b_gate)
                    with nc.allow_low_precision("s reduce in one pass"):
                        nc.vector.tensor_reduce(
                            out=s_g[:, 0:n_dve], in_=prod[:, 0:n_dve, :],
                            axis=mybir.AxisListType.X, op=mybir.AluOpType.add)
                    for j in range(n_dve, nrows):
                        nc.scalar.activation(
                            scratch_act, prod[:, j, :], Act.Copy,
                            accum_out=s_g[:, j : j + 1])
                else:
                    nc.vector.memset(s_g, 0.0)
                nc.scalar.activation(e_g, s_g, Act.Exp)
                # P_e = onehot * e  (batched broadcast multiply on the
                # otherwise-idle GPSIMD engine, one per group)
                pe_g = pepool.tile([P, BLK_PER_DMA, SEGW], bf16, tag="pe_g")
                if "tt" not in ablate:
                    e_ap = e_g[:, :]
                    e_3d = bass.AP(e_ap.tensor, e_ap.offset,
                                   [list(d) for d in e_ap.ap] + [[1, 1]])
                    a_oh, a_e = broadcast_tensor_aps(oh_g[:, :, :], e_3d)
                    nc.gpsimd.tensor_mul(pe_g[:, :, :], a_oh, a_e)
                else:
                    nc.vector.tensor_copy(pe_g, oh_g)
                vt_tiles[g] = vt
                pe_tiles[g] = pe_g

            gb = 0
            for w in range(W):
                segw = win_w[w]
                uw = psum2.tile([SEGW, EMBA], f32, tag="uw")
                for b in range(b_w[w]):
                    g, j = divmod(gb, BLK_PER_DMA)
                    ensure_group(g)
                    first = b == 0
                    last = b == b_w[w] - 1
                    if "mm" not in ablate or first or last:
                        nc.tensor.matmul(uw, lhsT=pe_tiles[g][:, j, :],
                                         rhs=vt_tiles[g][:, j, :],
                                         start=first, stop=last)
                    gb += 1
                # ---- window epilogue ----
                off = win_lo[w]
                u_sb = stpool.tile([SEGW, EMBA], f32, tag="u_sb")
                nc.scalar.copy(u_sb, uw)
                t0p = psum3.tile([P, SEGW], f32, tag="t0p")
                nc.tensor.transpose(t0p, u_sb[:, 0:HALF],
                                    ident_sb[0:SEGW, 0:SEGW])
                t1p = psum3.tile([P, SEGW], f32, tag="t1p")
                nc.tensor.transpose(t1p, u_sb[:, HALF:EMB],
                                    ident_sb[0:SEGW, 0:SEGW])
                nc.scalar.copy(u_stage0[:, off : off + segw], t0p[:, 0:segw])
                nc.scalar.copy(u_stage1[:, off : off + segw], t1p[:, 0:segw])
                nc.scalar.copy(d_cols[:, w : w + 1], u_sb[:, EMB:EMBA])

            # ---- D: [seg-in-window, window] -> [group, seg-in-group] rows
            # via DRAM roundtrip, then PE-transpose to per-partition layout.
            d_dram = dram.tile([SEGW, W], f32, tag="d_dram")
            nc.sync.dma_start(out=d_dram, in_=d_cols)
            d_rows = const.tile([P, GRP], f32, tag="d_rows")
            nc.vector.memset(d_rows, 0.0)
            nc.sync.dma_start(
                out=d_rows[0:n_grp, :].rearrange("g (a r) -> g a r", r=SEGW),
                in_=d_dram.rearrange("r (g a) -> g a r", g=n_grp),
            )
            dT = psumd.tile([P, P], f32, tag="dT")
            nc.tensor.transpose(dT, d_rows, ident_sb)
            d_sq = const.tile([P, n_grp], f32, tag="d_sq")
            nc.vector.tensor_copy(d_sq, dT[:, 0:n_grp])
            d_cl = const.tile([P, n_grp], f32, tag="d_cl")
            nc.vector.tensor_scalar_max(d_cl, d_sq, 1e-30)
            rec = const.tile([P, n_grp], f32, tag="rec")
            nc.vector.reciprocal(rec, d_cl)
            if debug_taps:
                nc.sync.dma_start(out=dbg_d[:, 0:n_grp], in_=d_sq)
                nc.sync.dma_start(out=dbg_d[:, 4:4+n_grp], in_=rec)
                nc.sync.dma_start(out=dbg_d[0:SEGW, 8:8+W], in_=d_cols)
                nc.sync.dma_start(out=dbg_d[44:44+n_grp, 128:256],
                                  in_=d_rows[0:n_grp, :])
                nc.sync.dma_start(out=dbg_d[48:80, 64:128],
                                  in_=u_stage0[0:32, 0:64])

            # ---- final: Z = U @ attn_w + D * attn_b, out = Z / D ----
            for g in range(n_grp):
                lo = g * GRP
                z = psum1.tile([GRP, EMB], f32, tag="z")
                nc.tensor.matmul(z, lhsT=u_stage0[:, lo : lo + GRP], rhs=attn0_sb,
                                 start=True, stop=False)
                nc.tensor.matmul(z, lhsT=u_stage1[:, lo : lo + GRP], rhs=attn1_sb,
                                 start=False, stop=True)
                o_sb = opool.tile([GRP, EMB], f32, tag="o_sb")
                nc.scalar.activation(o_sb, z, Act.Copy, scale=rec[:, g : g + 1])
                nc.vector.tensor_add(o_sb, o_sb, attnb_sb)
                nc.sync.dma_start(out=out_d[lo : lo + GRP, :], in_=o_sb)

        for _rep in range(reps):
            one_pass()

    nc.compile()
    return nc


def _get_program(meta, reps=1):
    key = (meta["W"], meta["b_w"], meta["win_lo"], meta["win_w"],
           meta["spc"], reps)
    if key not in _CACHE:
        _CACHE[key] = build_bass(meta, reps=reps)
    return _CACHE[key]


def make_const_inputs(gate_w, attn_w, attn_b):
    gate_rep = np.ascontiguousarray(
        np.broadcast_to(np.asarray(gate_w, np.float32).reshape(1, EMB),
                        (P, EMB))).astype(ml_dtypes.bfloat16)
    return {
        "gate_rep": gate_rep,
        "attn_w": np.asarray(attn_w, np.float32).astype(ml_dtypes.bfloat16),
        "attn_b": np.ascontiguousarray(np.broadcast_to(
            np.asarray(attn_b, np.float32).reshape(1, EMB), (P, EMB))),
        "ident": np.eye(P, dtype=np.float32),
    }


def build_in_maps(values, indices, num_graphs, gate_w, attn_w, attn_b):
    G = int(num_graphs)
    per_core, meta = prepare_host(values, indices, G)
    consts = make_const_inputs(gate_w, attn_w, attn_b)
    in_maps = [{**consts, "v": pc["v"], "oh": pc["oh"]} for pc in per_core]
    return in_maps, meta


# ----------------------------------------------------------------------------
# Public entry point.
# ----------------------------------------------------------------------------
def kernel(values, indices, num_graphs, gate_w, gate_b, attn_w, attn_b):
    from concourse.bass_utils import run_bass_kernel_spmd

    in_maps, meta = build_in_maps(values, indices, num_graphs,
                                  gate_w, attn_w, attn_b)
    nc = _get_program(meta)
    res = run_bass_kernel_spmd(nc, in_maps, core_ids=list(range(NCORES)))
    out = np.concatenate([res.results[c]["out"] for c in range(NCORES)], axis=0)
    return out[: int(num_graphs)]
